# revision 14
# baseline (speedup 1.0000x reference)
"""Trainium2 Bass kernel for nn_LogicLayer (ProductTNorm 'and' LogicLayer forward).

Math: y[b,o] = prod_i (1 - u[b,i]*v[o,i]),  u = 1-atoms, v = sigmoid(weights)
    = exp( sum_i ln(1 - u*v) )
    ~ exp( sum_k c_k * (F_k(u) @ G_k(v)^T) )        (separable approximation)

The coefficients are fitted offline against the TRUE device-computed feature
tensors (dumped once by kernel_feat.py), so all fp16 rounding and activation
table behavior is absorbed into the fit; see fit_exp7.py.

u-side features F(u): integer powers u^d (fp16 tensor-tensor chain on
VectorE, fixed addition-chain DAG shared with the feature dump), exp(-l*u) /
ln(1-s*u) (one ScalarE op each, straight from atoms), or ones.  v-side
G(v) = v^e for any real e: ONE ScalarE op Exp(-e*sp + ln|c|) with
sp = ln(1+exp(-w)) = -ln(v).  The activation-table chooser is pinned to the
combined exp/ln set -> exactly one ~1.3us table load.

Terms sharing a u-feature are MERGED: their stationaries are combined on
VectorE (add/subtract, signs live in the stationary), so each distinct
u-feature costs just 4 matmuls ([o-chunk 128, b 512] into 2 PSUM banks).
8 cores, data-parallel over batch (512 rows/core), weights replicated.
PSUM accumulates -S; final y = Exp(-psum + IN*c00) on ScalarE.  Dummy
warm-up matmuls keep the PE HAM clock-gate at full rate.
"""

import math
import os
from contextlib import ExitStack

import numpy as np

B, OUT, IN = 4096, 256, 256
NCORES = 8
B_LOC = B // NCORES          # 512 batch rows per core
NIT = IN // 128              # 2 i-tiles
NOC = OUT // 128             # 2 o-chunks
N_WARM_MM = 10

# Terms: (ufeat, vfeat, coef); fitted on true HW features (fit_exp7, n=18,
# lam_e=1e4), predicted y-space norm relerr 9.7e-3.
C00 = 0.002480384272617748
TERMS = [
    (("pow", 6), ("pow", 6.0), -0.6017309528802406),
    (("pow", 3), ("pow", 1.75), -0.5295253731554277),
    (("pow", 16), ("pow", 0.75), 0.12450786592834069),
    (("exp", 8.0), ("one",), -0.002659881747928481),
    (("pow", 1), ("pow", 1.0), -1.0527789152004556),
    (("pow", 2), ("pow", 4.5), -0.6383433418493705),
    (("pow", 16), ("one",), -0.04865297929294526),
    (("pow", 16), ("pow", 20), -1.7620718504028818),
    (("pow", 1), ("pow", 20), -0.2552480533614912),
    (("exp", 8.0), ("pow", 20), 0.10309838847443753),
    (("pow", 5), ("one",), 0.027213671877909178),
    (("pow", 16), ("pow", 3.0), -0.20508696934067444),
    (("pow", 6), ("pow", 20), 0.6568753238186564),
    (("pow", 1), ("pow", 11), 0.3389684903137648),
    (("exp", 3.0), ("pow", 15), -0.08163195286262581),
    (("exp", 8.0), ("pow", 4.5), 0.0072610493406959735),
    (("pow", 7), ("pow", 12), -0.5898690558651728),
    (("pow", 16), ("pow", 9), 0.21133024073407342),
]

_COMPILED = {}


def _addition_chain(targets):
    """Greedy addition chain covering targets; returns ordered (t, p, q)."""
    have = {1}
    steps = []

    def build(t):
        if t in have:
            return
        half = t // 2
        if t % 2 == 0 and half in have:
            steps.append((t, half, half)); have.add(t); return
        best = max((p for p in have if p < t), default=None)
        assert best is not None
        build(t - best)
        steps.append((t, best, t - best)); have.add(t)

    for t in sorted(set(targets)):
        build(t)
    return steps


def _term_layout():
    """Groups, bias-column and host-constant layout (shared with make_in_maps)."""
    groups = []          # (uf, [(vf, c), ...]) in stable uf-first-seen order
    by_uf = {}
    for uf, vf, c in TERMS:
        if uf not in by_uf:
            by_uf[uf] = []
            groups.append(uf)
        by_uf[uf].append((vf, c))
    # reorder inside each group: a negative-c pow-e term first if one exists
    # (its exp output IS the initial stationary, no sign fix needed)
    glist = []
    for uf in groups:
        ts = by_uf[uf]
        firsts = [t for t in ts if t[1] < 0 and t[0][0] == "pow"]
        if firsts:
            ts = [firsts[0]] + [t for t in ts if t is not firsts[0]]
        glist.append((uf, ts))
    # bias columns: one per pow-e term (ln|c|), then IN*C00, then usf biases
    pow_terms = []       # (uf, vf, c) in emission order
    for uf, ts in glist:
        for vf, c in ts:
            if vf[0] == "pow":
                pow_terms.append((uf, vf, c))
    ufeat_scalar = [uf for uf in groups if uf[0] in ("log", "exp")]
    e0_terms = []
    for uf, ts in glist:
        for vf, c in ts:
            if vf[0] == "one":
                e0_terms.append((uf, vf, c))
    ncb = len(pow_terms) + 1 + len(ufeat_scalar)
    return glist, pow_terms, ufeat_scalar, e0_terms, ncb


def _patch_act_tables():
    """Pin the activation table-set chooser to natural_log_exp_and_others
    (contains both exp and ln) -> exactly ONE ACT_TABLE_LOAD."""
    import concourse.bacc as bacc
    from concourse import hw_specs

    if getattr(bacc, "_act_tables_combined_patch", False):
        return
    orig = hw_specs.get_activation_tables

    def combined_only(arch):
        tabs = orig(arch)
        keep = "natural_log_exp_and_others"
        if keep in tabs:
            tabs = {k: (vs if k == keep else set()) for k, vs in tabs.items()}
        return tabs

    bacc.get_activation_tables = combined_only
    bacc._act_tables_combined_patch = True


def _build_nc():
    import concourse.bacc as bacc
    import concourse.mybir as mybir
    import concourse.tile as tile

    _patch_act_tables()

    AF = mybir.ActivationFunctionType
    ALU = mybir.AluOpType
    F32 = mybir.dt.float32
    F16 = mybir.dt.float16

    nc = bacc.Bacc(
        "TRN2", target_bir_lowering=False, debug=False, num_devices=NCORES
    )

    glist, pow_terms, ufeat_scalar, e0_terms, ncb = _term_layout()

    aT = nc.dram_tensor("aT", [IN, B_LOC], F32, kind="ExternalInput").ap()
    wT = nc.dram_tensor("wT", [IN, OUT], F32, kind="ExternalInput").ap()
    cbias = nc.dram_tensor("cbias", [128, ncb], F32, kind="ExternalInput").ap()
    cmov = nc.dram_tensor("cmov", [128, B_LOC], F16, kind="ExternalInput").ap()
    n_e0 = len(e0_terms)
    cstat = (
        nc.dram_tensor("cstat", [128, n_e0 * NIT * OUT], F16, kind="ExternalInput").ap()
        if n_e0
        else None
    )
    y = nc.dram_tensor("y", [OUT, B_LOC], F32, kind="ExternalOutput").ap()

    # fixed addition-chain DAG (shared with kernel_feat dump), ancestor
    # closure of the selected powers
    pow_ds = sorted({uf[1] for uf, _ in glist if uf[0] == "pow"})
    full = _addition_chain(list(range(1, 17)))
    parents = {t: (p, q) for t, p, q in full}
    need = set()

    def _close(d):
        if d == 1 or d in need:
            return
        need.add(d)
        p, q = parents[d]
        _close(p); _close(q)

    for d in pow_ds:
        _close(d)
    chain = [(t, p, q) for (t, p, q) in full if t in need]
    chain_idx = {1: 0}
    for i, (t, _, _) in enumerate(chain):
        chain_idx[t] = i + 1

    def avail(uf):
        if uf[0] == "pow":
            return chain_idx.get(uf[1], 99)
        return -1

    gorder = sorted(range(len(glist)), key=lambda i: avail(glist[i][0]))

    with tile.TileContext(nc) as tc, ExitStack() as es:
        const = es.enter_context(tc.tile_pool(name="const", bufs=1))
        ps_pool = es.enter_context(tc.tile_pool(name="ps", bufs=1, space="PSUM"))

        # --- warm activation (pulls the single exp/ln table-set load to t~0)
        warm = const.tile([128, 1], F32, name="warm", tag="warm")
        nc.vector.memset(warm[:], 1.0)
        warm2 = const.tile([128, 1], F32, name="warm2", tag="warm2")
        nc.scalar.activation(warm2[:], warm[:], AF.Exp)

        # --- dummy matmuls keep the PE HAM clock warm before the real stream
        g_stat = const.tile([128, 128], F16, name="g_stat", tag="g_stat")
        g_mov = const.tile([128, B_LOC], F16, name="g_mov", tag="g_mov")
        nc.vector.memset(g_stat[:], 0.0)
        nc.vector.memset(g_mov[:], 0.0)
        ps_warm = ps_pool.tile([128, B_LOC], F32, name="ps_warm", tag="ps_warm")
        for _ in range(N_WARM_MM):
            nc.tensor.matmul(ps_warm[:], lhsT=g_stat[:], rhs=g_mov[:],
                             start=True, stop=True)

        # --- input DMAs: atoms -> sync queue, weights+consts -> gpsimd queue
        w_sb = const.tile([128, NIT * OUT], F32, name="w_sb", tag="w_sb")
        for it in range(NIT):
            nc.gpsimd.dma_start(
                w_sb[:, it * OUT : (it + 1) * OUT],
                wT[it * 128 : (it + 1) * 128, :],
            )
        ACH = B_LOC // 2
        a_sb = const.tile([128, NIT * B_LOC], F32, name="a_sb", tag="a_sb")
        for it in range(NIT):
            for q in range(2):
                nc.sync.dma_start(
                    a_sb[:, it * B_LOC + q * ACH : it * B_LOC + (q + 1) * ACH],
                    aT[it * 128 : (it + 1) * 128, q * ACH : (q + 1) * ACH],
                )
        cb_sb = const.tile([128, ncb], F32, name="cb_sb", tag="cb_sb")
        nc.gpsimd.dma_start(cb_sb[:], cbias[:])
        cm_sb = const.tile([128, B_LOC], F16, name="cm_sb", tag="cm_sb")
        nc.gpsimd.dma_start(cm_sb[:], cmov[:])
        if n_e0:
            cs_sb = const.tile([128, n_e0 * NIT * OUT], F16, name="cs_sb", tag="cs_sb")
            nc.gpsimd.dma_start(cs_sb[:], cstat[:])

        # --- sp = ln(1+e^-w) on ScalarE
        t_sb = const.tile([128, NIT * OUT], F32, name="t_sb", tag="t_sb")
        sp_sb = const.tile([128, NIT * OUT], F32, name="sp_sb", tag="sp_sb")
        nc.scalar.activation(t_sb[:], w_sb[:], AF.Exp, scale=-1.0)
        nc.scalar.activation(sp_sb[:], t_sb[:], AF.Ln, bias=1.0)

        # --- u features
        u_tiles = {}
        u1 = const.tile([128, NIT * B_LOC], F16, name="u_pow1", tag="u_pow1")
        nc.vector.tensor_scalar(u1[:], a_sb[:], -1.0, 1.0, ALU.mult, ALU.add)
        u_tiles[("pow", 1)] = u1
        for (t, p, q) in chain:
            ut = const.tile(
                [128, NIT * B_LOC], F16, name=f"u_pow{t}", tag=f"u_pow{t}"
            )
            nc.vector.tensor_tensor(
                ut[:], u_tiles[("pow", p)][:], u_tiles[("pow", q)][:], ALU.mult
            )
            u_tiles[("pow", t)] = ut
        for k, uf in enumerate(ufeat_scalar):
            ut = const.tile(
                [128, NIT * B_LOC], F16, name=f"u_sf{k}", tag=f"u_sf{k}"
            )
            bcol = cb_sb[:, len(pow_terms) + 1 + k : len(pow_terms) + 2 + k]
            fn = AF.Ln if uf[0] == "log" else AF.Exp
            nc.scalar.activation(ut[:], a_sb[:], fn, scale=float(uf[1]), bias=bcol)
            u_tiles[uf] = ut

        # --- merged stationaries: W_g = sum_t (-c_t) * v^{e_t}
        # pow-e parts come from ScalarE Exp(-e*sp + ln|c|) (positive);
        # "one" parts come from host constants (cstat holds (-c) directly).
        pt_index = {}
        for idx, (uf, vf, c) in enumerate(pow_terms):
            pt_index[(uf, tuple(vf), c)] = idx
        e0_index = {}
        for idx, (uf, vf, c) in enumerate(e0_terms):
            e0_index[(uf, tuple(vf), c)] = idx

        vp_pool = es.enter_context(tc.tile_pool(name="vp", bufs=3))
        acc_pool = es.enter_context(tc.tile_pool(name="acc", bufs=2))
        w_tiles = {}
        for gi in gorder:
            uf, ts = glist[gi]
            wt_tile = const.tile(
                [128, NIT * OUT], F16, name=f"W_{gi}", tag=f"W_{gi}"
            )
            if len(ts) == 1 and ts[0][0][0] == "pow" and ts[0][1] < 0:
                vf, c = ts[0]
                idx = pt_index[(uf, tuple(vf), c)]
                nc.scalar.activation(
                    wt_tile[:], sp_sb[:], AF.Exp, scale=-float(vf[1]),
                    bias=cb_sb[:, idx : idx + 1],
                )
                w_tiles[gi] = wt_tile
                continue
            # multi-term (or positive-first) groups: accumulate in fp32 on
            # DVE, single fp16 cast at the end (avoids fp16 cancellation)
            acc = acc_pool.tile([128, NIT * OUT], F32, name="acc", tag="acc")
            first = True
            for vf, c in ts:
                if vf[0] == "pow":
                    idx = pt_index[(uf, tuple(vf), c)]
                    pt = vp_pool.tile([128, NIT * OUT], F32, name="vp", tag="vp")
                    nc.scalar.activation(
                        pt[:], sp_sb[:], AF.Exp, scale=-float(vf[1]),
                        bias=cb_sb[:, idx : idx + 1],
                    )
                    if first:
                        if c < 0:
                            nc.vector.tensor_copy(acc[:], pt[:])
                        else:
                            nc.vector.tensor_scalar_mul(acc[:], pt[:], -1.0)
                    else:
                        nc.vector.tensor_tensor(
                            acc[:], acc[:], pt[:],
                            ALU.subtract if c > 0 else ALU.add,
                        )
                else:
                    idx = e0_index[(uf, tuple(vf), c)]
                    sl = cs_sb[:, idx * NIT * OUT : (idx + 1) * NIT * OUT]
                    if first:
                        nc.vector.tensor_copy(acc[:], sl)
                    else:
                        nc.vector.tensor_tensor(acc[:], acc[:], sl, ALU.add)
                first = False
            nc.vector.tensor_copy(wt_tile[:], acc[:])
            w_tiles[gi] = wt_tile

        # --- matmuls: psum = -S (2 banks, 4 MMs per group)
        psum = [
            ps_pool.tile([128, B_LOC], F32, name=f"psum{oc}", tag=f"psum{oc}")
            for oc in range(NOC)
        ]
        nmm = len(gorder) * NIT
        k = 0
        for gi in gorder:
            uf, ts = glist[gi]
            for it in range(NIT):
                k += 1
                if uf[0] == "one":
                    mov = cm_sb[:, :]
                else:
                    mov = u_tiles[uf][:, it * B_LOC : (it + 1) * B_LOC]
                for oc in range(NOC):
                    nc.tensor.matmul(
                        psum[oc][:, :],
                        lhsT=w_tiles[gi][:, it * OUT + oc * 128 : it * OUT + (oc + 1) * 128],
                        rhs=mov,
                        start=(k == 1),
                        stop=(k == nmm),
                    )

        # --- finale: y = Exp(-psum + IN*C00)
        YCH = B_LOC // 2
        y_sb = const.tile([128, NOC * B_LOC], F32, name="y_sb", tag="y_sb")
        for oc in range(NOC):
            for qch in range(2):
                sl = slice(qch * YCH, (qch + 1) * YCH)
                osl = slice(oc * B_LOC + qch * YCH, oc * B_LOC + (qch + 1) * YCH)
                nc.scalar.activation(
                    y_sb[:, osl], psum[oc][:, sl], AF.Exp,
                    scale=-1.0, bias=cb_sb[:, len(pow_terms) : len(pow_terms) + 1],
                )
                nc.sync.dma_start(y[oc * 128 : (oc + 1) * 128, sl], y_sb[:, osl])

    nc.compile()
    return nc


def get_nc():
    if "nc" not in _COMPILED:
        _COMPILED["nc"] = _build_nc()
    return _COMPILED["nc"]


def _host_consts():
    glist, pow_terms, ufeat_scalar, e0_terms, ncb = _term_layout()
    cbias = np.empty((128, ncb), np.float32)
    for idx, (uf, vf, c) in enumerate(pow_terms):
        cbias[:, idx] = math.log(abs(c))
    cbias[:, len(pow_terms)] = IN * C00
    for k, uf in enumerate(ufeat_scalar):
        cbias[:, len(pow_terms) + 1 + k] = (
            1.0 - uf[1] if uf[0] == "log" else -uf[1]
        )
    cmov = np.ones((128, B_LOC), np.float16)
    n_e0 = len(e0_terms)
    cstat = np.empty((128, max(1, n_e0) * NIT * OUT), np.float16)
    for idx, (uf, vf, c) in enumerate(e0_terms):
        cstat[:, idx * NIT * OUT : (idx + 1) * NIT * OUT] = -c
    return cbias, cmov, cstat, n_e0


def make_in_maps(atoms: np.ndarray, weights: np.ndarray):
    atoms = np.asarray(atoms)
    weights = np.asarray(weights)
    aT = np.ascontiguousarray(atoms.T.astype(np.float32, copy=False))
    wT = np.ascontiguousarray(weights.T.astype(np.float32, copy=False))
    cbias, cmov, cstat, n_e0 = _host_consts()
    in_maps = []
    for c in range(NCORES):
        a_loc = np.ascontiguousarray(aT[:, c * B_LOC : (c + 1) * B_LOC])
        m = {"aT": a_loc, "wT": wT, "cbias": cbias, "cmov": cmov}
        if n_e0:
            m["cstat"] = cstat
        in_maps.append(m)
    return in_maps


def run(atoms: np.ndarray, weights: np.ndarray, **spmd_kwargs):
    from concourse.bass_utils import run_bass_kernel_spmd

    nc = get_nc()
    in_maps = make_in_maps(atoms, weights)
    res = run_bass_kernel_spmd(nc, in_maps, core_ids=list(range(NCORES)), **spmd_kwargs)
    yT = np.concatenate([res.results[c]["y"] for c in range(NCORES)], axis=1)
    out = np.ascontiguousarray(yT.T).astype(np.float32, copy=False)
    return out, res


def kernel(atoms: np.ndarray, weights: np.ndarray) -> np.ndarray:
    out, _ = run(atoms, weights)
    return out
